# revision 16
# baseline (speedup 1.0000x reference)
"""GCN layer (x@Wn aggregated over edges + x@Ws + bias) on 8 Trainium2 cores.

V4: quad-row gather. The Q7 SWDGE descriptor loop costs ~8ns per scanned
index slot, so wall time ~= total gather slots. Gathering 1024B elements
(4 consecutive bf16 node rows, idx = src>>2 < 25000, int16-safe) removes
the need for src chunks entirely, so edge groups are whole dst TILES
(lambda ~2042) instead of (tile, chunk) cells (lambda ~510): the
max-over-cores + ceil-128 padding tax drops from ~49k to ~13k slots/core
(scan 249.5k -> ~214k).

The wanted row within each gathered quad is q = src & 3. Edges are sorted
q-major within their tile, so 128-slot blocks are q-pure except near the
3 q-run boundaries (whose positions differ per core). The shared program
emits one matmul per (block, q) pair occurring in ANY core ("pieces"),
with lhsT = the q-th 128-col sub-slice of the block's gathered quad data
and S built from a per-piece w column that each core masks to its own
edges of that q (w=0 elsewhere, including all padding slots).

Per tile: psumA += Xg_piece.T @ S_piece over pieces, then
psumB = aggT.T @ W_nbrs + xT_tile.T @ W_self + ones.T @ bias -> out.
iota for the S-build lives in PSUM f32 (keeps DVE in 1x single-port
mode: no GpSimd SBUF-port starvation).
"""
import sys

sys.path.insert(0, "/opt/trn_rl_repo")

import numpy as np
import ml_dtypes

import concourse.bacc as bacc
import concourse.mybir as mybir
from concourse.bass_utils import run_bass_kernel_spmd
from concourse.tile import TileContext

BF16 = mybir.dt.bfloat16
F32 = mybir.dt.float32
I16 = mybir.dt.int16
nbf = ml_dtypes.bfloat16

N = 100000
E = 1600000
D = 128
NC = 8
NPC = N // NC              # 12500 nodes per core
TPC = (NPC + 127) // 128   # 98 dst tiles per core
NPAD = TPC * 128           # 12544 padded nodes per core
QROWS = N // 4             # 25000 quad rows (< 2**15, int16-safe)
QELEM = 4 * D              # 512 bf16 = 1024B per gathered element
MAXCOMP = 16               # largest gather component (blocks = 2048 idxs)


def _pow2_components(nblk):
    comps = []
    while nblk >= MAXCOMP:
        comps.append(MAXCOMP)
        nblk -= MAXCOMP
    b = MAXCOMP // 2
    while nblk > 0 and b >= 1:
        if nblk >= b:
            comps.append(b)
            nblk -= b
        b //= 2
    return comps


def _balance_dsts(dst):
    """Assign dst nodes to (core, tile, slot) bins, balancing per-bin edge
    counts (sum of degrees) via capacity-constrained LPT so nearly every
    (core, tile) bin lands at <= 2048 edges => 16 blocks, no cross-core
    max tax. Output is unpermuted on the host, so the assignment is free.
    Returns core_of, tile_of, slot_of, members[c][t] = node list."""
    import heapq

    deg = np.bincount(dst, minlength=N)
    order = np.argsort(-deg, kind="stable")
    NB_ = NC * TPC
    heap = [(0, b) for b in range(NB_)]
    heapq.heapify(heap)
    cap = np.full(NB_, 128, dtype=np.int64)
    core_of = np.empty(N, dtype=np.int64)
    tile_of = np.empty(N, dtype=np.int64)
    slot_of = np.empty(N, dtype=np.int64)
    members = [[[] for _ in range(TPC)] for _ in range(NC)]
    for nid in order:
        s, b = heapq.heappop(heap)
        c, t = divmod(b, TPC)
        core_of[nid] = c
        tile_of[nid] = t
        slot_of[nid] = 128 - cap[b]
        members[c][t].append(nid)
        cap[b] -= 1
        s += int(deg[nid])
        if cap[b] > 0:
            heapq.heappush(heap, (s, b))
    return core_of, tile_of, slot_of, members


def _preprocess(edge_src, edge_dst, edge_weight):
    src = np.asarray(edge_src, dtype=np.int64)
    dst = np.asarray(edge_dst, dtype=np.int64)
    wgt = np.asarray(edge_weight, dtype=np.float32)

    core_of, tile_of, slot_of, members = _balance_dsts(dst)
    core = core_of[dst]
    tile = tile_of[dst]

    counts = np.zeros((NC, TPC), dtype=np.int64)
    np.add.at(counts, (core, tile), 1)
    B = np.maximum((-(-counts // 128)).max(axis=0), 1)   # [TPC]
    NBLK = int(B.sum())
    blkoff = np.zeros(TPC, dtype=np.int64)
    off = 0
    for t in range(TPC):
        blkoff[t] = off
        off += int(B[t])
    BMAX = int(B.max())

    # gather components per tile: (tile, rel_block, nblocks, idx_col0)
    comps = []
    tile_comp0 = np.zeros(TPC + 1, dtype=np.int64)
    icol = 0
    for t in range(TPC):
        tile_comp0[t] = len(comps)
        rel = 0
        for nb in _pow2_components(int(B[t])):
            comps.append((t, rel, nb, icol))
            rel += nb
            icol += max(nb * 8, 64)   # 64-col (128B) aligned idx regions
        assert rel == int(B[t])
    tile_comp0[TPC] = len(comps)
    NICOL = icol
    # idx head/tail split: first few tiles' indices load in a small early
    # DMA so the first gather doesn't wait on the full idx table
    KCOL = comps[int(tile_comp0[6])][3]

    # per-core slot arrays
    idx_blk_all = []
    w_all = []
    dl_all = []
    qv_all = np.full((NC, NBLK * 128), -1, dtype=np.int8)
    for c in range(NC):
        sel = core == c
        t_c = tile[sel]
        s_c = src[sel]
        d_c = slot_of[dst[sel]]
        w_c = wgt[sel]
        q_c = (s_c & 3).astype(np.int8)
        o = np.lexsort((q_c, t_c))      # tile-major, q-minor
        t_c, s_c, d_c, w_c, q_c = t_c[o], s_c[o], d_c[o], w_c[o], q_c[o]

        idx_blk = np.zeros(NBLK * 128, dtype=np.int16)
        wf = np.zeros(NBLK * 128, dtype=np.float32)
        dlf = np.zeros(NBLK * 128, dtype=np.float32)
        cnt = counts[c]
        pos = 0
        for t in range(TPC):
            n = int(cnt[t])
            if n:
                slot0 = int(blkoff[t]) * 128
                idx_blk[slot0 : slot0 + n] = (s_c[pos : pos + n] >> 2)
                wf[slot0 : slot0 + n] = w_c[pos : pos + n]
                dlf[slot0 : slot0 + n] = d_c[pos : pos + n]
                qv_all[c, slot0 : slot0 + n] = q_c[pos : pos + n]
                pos += n
        assert pos == int(sel.sum())
        idx_blk_all.append(idx_blk)
        w_all.append(wf)
        dl_all.append(dlf)

    # pieces: per (tile, block): q values present in any core
    pieces = []            # (t, b, q, gb)
    tile_piece0 = np.zeros(TPC + 1, dtype=np.int64)
    for t in range(TPC):
        tile_piece0[t] = len(pieces)
        for b in range(int(B[t])):
            gb = int(blkoff[t]) + b
            qs = qv_all[:, gb * 128 : (gb + 1) * 128]
            present = np.unique(qs[qs >= 0])
            if len(present) == 0:
                present = [0]   # all-padding block (w=0 everywhere)
            for q in present:
                pieces.append((t, b, int(q), gb))
    tile_piece0[TPC] = len(pieces)
    NPIECE = len(pieces)

    per_core = []
    for c in range(NC):
        idx_blk = idx_blk_all[c]
        wf = w_all[c]
        dlf = dl_all[c]
        qv = qv_all[c]

        # idx storage: per component, 64-col-aligned regions (wrapped x8)
        idx_cols = np.zeros((NICOL, 16), dtype=np.int16)
        for (t, rel, nb, col0) in comps:
            blk0 = int(blkoff[t]) + rel
            flat = idx_blk[blk0 * 128 : (blk0 + nb) * 128]
            idx_cols[col0 : col0 + nb * 8] = flat.reshape(-1, 16)
        idx_w = np.tile(idx_cols.T, (8, 1)).copy()        # [128, NICOL]

        wp = np.zeros((NPIECE, 128), dtype=np.float32)
        for pid, (t, b, q, gb) in enumerate(pieces):
            s0 = gb * 128
            mask = qv[s0 : s0 + 128] == q
            wp[pid] = wf[s0 : s0 + 128] * mask
        wp = wp.T.copy()                                  # [128, NPIECE]
        dl_pb = dlf.reshape(NBLK, 128).T.copy()           # [128, NBLK]
        per_core.append((idx_w, wp, dl_pb))

    meta = dict(
        B=B, NBLK=NBLK, NICOL=NICOL, NPIECE=NPIECE, BMAX=BMAX, KCOL=KCOL,
        blkoff=blkoff, comps=comps, tile_comp0=tile_comp0,
        pieces=pieces, tile_piece0=tile_piece0,
        core_of=core_of, tile_of=tile_of, slot_of=slot_of,
    )
    return meta, per_core


def _build_program(meta):
    B = meta["B"]
    NBLK = meta["NBLK"]
    NICOL = meta["NICOL"]
    NPIECE = meta["NPIECE"]
    BMAX = meta["BMAX"]
    blkoff = meta["blkoff"]
    comps = meta["comps"]
    tile_comp0 = meta["tile_comp0"]
    pieces = meta["pieces"]
    tile_piece0 = meta["tile_piece0"]

    KCOL = meta["KCOL"]

    nc = bacc.Bacc()
    # x as quad rows: row r = x[4r : 4r+4] flattened (same bytes as [N, D])
    xq_d = nc.declare_dram_parameter("xq", [QROWS, QELEM], BF16, isOutput=False)
    idxh_d = nc.declare_dram_parameter("idxh", [128, KCOL], I16, isOutput=False)
    idx_d = nc.declare_dram_parameter("idx", [128, NICOL - KCOL], I16, isOutput=False)
    wp_d = nc.declare_dram_parameter("wp", [128, NPIECE], F32, isOutput=False)
    dl_d = nc.declare_dram_parameter("dl", [128, NBLK], F32, isOutput=False)
    iota_d = nc.declare_dram_parameter("iota", [128, 128], F32, isOutput=False)
    wn_d = nc.declare_dram_parameter("wn", [128, 128], BF16, isOutput=False)
    ws_d = nc.declare_dram_parameter("ws", [128, 128], BF16, isOutput=False)
    xt_d = nc.declare_dram_parameter("xt", [128, NPAD], BF16, isOutput=False)
    bias_d = nc.declare_dram_parameter("bias_bc", [128, 128], F32, isOutput=False)
    out_d = nc.declare_dram_parameter("out", [NPAD, 128], F32, isOutput=True)

    with TileContext(nc) as tc:
        with (
            tc.tile_pool(name="const", bufs=1) as cpool,
            tc.tile_pool(name="gather", bufs=3) as gpool,
            tc.tile_pool(name="work", bufs=4) as wpool,
            tc.tile_pool(name="outp", bufs=3) as opool,
            tc.tile_pool(name="psA", bufs=2, space="PSUM") as pApool,
            tc.tile_pool(name="psB", bufs=2, space="PSUM") as pBpool,
            tc.tile_pool(name="psI", bufs=1, space="PSUM") as pIpool,
        ):
            # warm up the gpsimd gather ucode (LOAD_LIB + ~6us IRAM load)
            # before the idx DMAs land, so the first real gather isn't gated
            warm_idx = cpool.tile([128, 64], I16)
            nc.vector.memset(warm_idx[:], 0)
            warm_out = cpool.tile([128, QELEM], BF16)
            nc.gpsimd.dma_gather(
                out_ap=warm_out[:].rearrange("p (b e) -> p b e", e=QELEM),
                in_ap=xq_d[:, :],
                idxs_ap=warm_idx[:, 0:8],
                num_idxs=128,
                num_idxs_reg=128,
                elem_size=QELEM,
                single_packet=False,
            )
            idxh_t = cpool.tile([128, KCOL], I16)
            nc.sync.dma_start(out=idxh_t[:], in_=idxh_d[:])
            idx_t = cpool.tile([128, NICOL - KCOL], I16)
            nc.sync.dma_start(out=idx_t[:], in_=idx_d[:])
            wp_t = cpool.tile([128, NPIECE], F32)
            nc.sync.dma_start(out=wp_t[:], in_=wp_d[:])
            dl_t = cpool.tile([128, NBLK], F32)
            nc.sync.dma_start(out=dl_t[:], in_=dl_d[:])
            iota_sb = cpool.tile([128, 128], F32)
            nc.sync.dma_start(out=iota_sb[:], in_=iota_d[:])
            wn_t = cpool.tile([128, 128], BF16)
            nc.sync.dma_start(out=wn_t[:], in_=wn_d[:])
            ws_t = cpool.tile([128, 128], BF16)
            nc.sync.dma_start(out=ws_t[:], in_=ws_d[:])
            xt_t = cpool.tile([128, NPAD], BF16)
            nc.sync.dma_start(out=xt_t[:], in_=xt_d[:])
            bias_t = cpool.tile([128, 128], F32)
            nc.sync.dma_start(out=bias_t[:], in_=bias_d[:])
            bias_bf = cpool.tile([1, 128], BF16)
            nc.vector.tensor_copy(out=bias_bf[:], in_=bias_t[0:1, :])
            ones_t = cpool.tile([1, 128], BF16)
            nc.vector.memset(ones_t[:], 1.0)

            iota_ps = pIpool.tile([128, 128], F32, space="PSUM")
            nc.vector.tensor_copy(out=iota_ps[:], in_=iota_sb[:])

            tile_gbuf = {}

            def issue_tile_gathers(t):
                gt = gpool.tile([128, BMAX * QELEM], BF16, tag="g")
                for ci in range(int(tile_comp0[t]), int(tile_comp0[t + 1])):
                    _, rel, nb, col0 = comps[ci]
                    if col0 < KCOL:
                        iap = idxh_t[:, col0 : col0 + nb * 8]
                    else:
                        iap = idx_t[:, col0 - KCOL : col0 - KCOL + nb * 8]
                    nc.gpsimd.dma_gather(
                        out_ap=gt[:, rel * QELEM : (rel + nb) * QELEM]
                        .rearrange("p (b e) -> p b e", e=QELEM),
                        in_ap=xq_d[:, :],
                        idxs_ap=iap,
                        num_idxs=nb * 128,
                        num_idxs_reg=nb * 128,
                        elem_size=QELEM,
                        single_packet=False,
                    )
                tile_gbuf[t] = gt

            for t in range(TPC):
                while len(tile_gbuf) < min(t + 3, TPC):
                    issue_tile_gathers(len(tile_gbuf))

                gt = tile_gbuf[t]
                p0, p1 = int(tile_piece0[t]), int(tile_piece0[t + 1])
                psumA = pApool.tile([128, 128], F32, space="PSUM", tag="psA")
                for i, pid in enumerate(range(p0, p1)):
                    _, b, q, gb = pieces[pid]
                    s_t = wpool.tile([128, 128], BF16, tag="sel")
                    nc.vector.tensor_scalar(
                        out=s_t[:],
                        in0=iota_ps[:],
                        scalar1=dl_t[:, gb : gb + 1],
                        scalar2=wp_t[:, pid : pid + 1],
                        op0=mybir.AluOpType.is_equal,
                        op1=mybir.AluOpType.mult,
                    )
                    nc.tensor.matmul(
                        out=psumA[:],
                        lhsT=gt[:, b * QELEM + q * 128 : b * QELEM + (q + 1) * 128],
                        rhs=s_t[:],
                        start=(i == 0),
                        stop=(pid == p1 - 1),
                    )
                aggT = wpool.tile([128, 128], BF16, tag="aggT")
                nc.scalar.copy(out=aggT[:], in_=psumA[:])
                psumB = pBpool.tile([128, 128], F32, space="PSUM", tag="psB")
                nc.tensor.matmul(
                    out=psumB[:], lhsT=aggT[:], rhs=wn_t[:],
                    start=True, stop=False,
                )
                nc.tensor.matmul(
                    out=psumB[:],
                    lhsT=xt_t[:, t * 128 : (t + 1) * 128],
                    rhs=ws_t[:],
                    start=False, stop=False,
                )
                nc.tensor.matmul(
                    out=psumB[:], lhsT=ones_t[:], rhs=bias_bf[:],
                    start=False, stop=True,
                )
                out_t = opool.tile([128, 128], F32, tag="out")
                nc.scalar.copy(out=out_t[:], in_=psumB[:])
                nc.sync.dma_start(
                    out=out_d[t * 128 : (t + 1) * 128, :], in_=out_t[:]
                )

    nc.compile()
    return nc


def kernel(x, edge_src, edge_dst, edge_weight, W_nbrs, W_self, bias, _trace=False,
           _tmpdir=None):
    x = np.asarray(x, dtype=np.float32)
    meta, per_core = _preprocess(edge_src, edge_dst, edge_weight)
    nc = _build_program(meta)

    xq = x.astype(nbf).reshape(QROWS, QELEM)
    iota = np.broadcast_to(np.arange(128, dtype=np.float32), (128, 128)).copy()
    wn = np.asarray(W_nbrs, dtype=np.float32).astype(nbf)
    ws = np.asarray(W_self, dtype=np.float32).astype(nbf)
    bias_bc = np.broadcast_to(np.asarray(bias, dtype=np.float32), (128, 128)).copy()

    core_of = meta["core_of"]
    tile_of = meta["tile_of"]
    slot_of = meta["slot_of"]
    in_maps = []
    for c in range(NC):
        idx_w, wp, dl_pb = per_core[c]
        nodes = np.where(core_of == c)[0]
        xt = np.zeros((128, NPAD), dtype=np.float32)
        xt[:, tile_of[nodes] * 128 + slot_of[nodes]] = x[nodes].T
        in_maps.append(
            dict(
                xq=xq,
                idxh=idx_w[:, : meta["KCOL"]].copy(),
                idx=idx_w[:, meta["KCOL"] :].copy(),
                wp=wp,
                dl=dl_pb,
                iota=iota,
                wn=wn,
                ws=ws,
                xt=xt.astype(nbf),
                bias_bc=bias_bc,
            )
        )

    res = run_bass_kernel_spmd(
        nc, in_maps, list(range(NC)), trace=_trace, tmpdir=_tmpdir
    )
    out = np.empty((N, D), dtype=np.float32)
    for c in range(NC):
        nodes = np.where(core_of == c)[0]
        out[nodes] = res.results[c]["out"][tile_of[nodes] * 128 + slot_of[nodes]]
    if _trace:
        kernel._last_result = res
    return out


# revision 19
# speedup vs baseline: 1.1972x; 1.1972x over previous
"""GCN layer (x@Wn aggregated over edges + x@Ws + bias) on 8 Trainium2 cores.

V4: quad-row gather. The Q7 SWDGE descriptor loop costs ~8ns per scanned
index slot, so wall time ~= total gather slots. Gathering 1024B elements
(4 consecutive bf16 node rows, idx = src>>2 < 25000, int16-safe) removes
the need for src chunks entirely, so edge groups are whole dst TILES
(lambda ~2042) instead of (tile, chunk) cells (lambda ~510): the
max-over-cores + ceil-128 padding tax drops from ~49k to ~13k slots/core
(scan 249.5k -> ~214k).

The wanted row within each gathered quad is q = src & 3. Edges are sorted
q-major within their tile, so 128-slot blocks are q-pure except near the
3 q-run boundaries (whose positions differ per core). The shared program
emits one matmul per (block, q) pair occurring in ANY core ("pieces"),
with lhsT = the q-th 128-col sub-slice of the block's gathered quad data
and S built from a per-piece w column that each core masks to its own
edges of that q (w=0 elsewhere, including all padding slots).

Per tile: psumA += Xg_piece.T @ S_piece over pieces, then
psumB = aggT.T @ W_nbrs + xT_tile.T @ W_self + ones.T @ bias -> out.
iota for the S-build lives in PSUM f32 (keeps DVE in 1x single-port
mode: no GpSimd SBUF-port starvation).
"""
import sys

sys.path.insert(0, "/opt/trn_rl_repo")

import numpy as np
import ml_dtypes

import concourse.bacc as bacc
import concourse.mybir as mybir
from concourse.bass_utils import run_bass_kernel_spmd
from concourse.tile import TileContext

BF16 = mybir.dt.bfloat16
F32 = mybir.dt.float32
I16 = mybir.dt.int16
nbf = ml_dtypes.bfloat16

N = 100000
E = 1600000
D = 128
NC = 8
NPC = N // NC              # 12500 nodes per core
TPC = (NPC + 127) // 128   # 98 dst tiles per core
NPAD = TPC * 128           # 12544 padded nodes per core
QROWS = N // 4             # 25000 quad rows (< 2**15, int16-safe)
QELEM = 4 * D              # 512 bf16 = 1024B per gathered element
MAXCOMP = 16               # largest gather component (blocks = 2048 idxs)


def _pow2_components(nblk):
    comps = []
    while nblk >= MAXCOMP:
        comps.append(MAXCOMP)
        nblk -= MAXCOMP
    b = MAXCOMP // 2
    while nblk > 0 and b >= 1:
        if nblk >= b:
            comps.append(b)
            nblk -= b
        b //= 2
    return comps


def _balance_dsts(dst):
    """Assign dst nodes to (core, tile, slot) bins, balancing per-bin edge
    counts (sum of degrees) via capacity-constrained LPT so nearly every
    (core, tile) bin lands at <= 2048 edges => 16 blocks, no cross-core
    max tax. Output is unpermuted on the host, so the assignment is free.
    Returns core_of, tile_of, slot_of, members[c][t] = node list."""
    import heapq

    deg = np.bincount(dst, minlength=N)
    order = np.argsort(-deg, kind="stable")
    NB_ = NC * TPC
    heap = [(0, b) for b in range(NB_)]
    heapq.heapify(heap)
    cap = np.full(NB_, 128, dtype=np.int64)
    core_of = np.empty(N, dtype=np.int64)
    tile_of = np.empty(N, dtype=np.int64)
    slot_of = np.empty(N, dtype=np.int64)
    members = [[[] for _ in range(TPC)] for _ in range(NC)]
    for nid in order:
        s, b = heapq.heappop(heap)
        c, t = divmod(b, TPC)
        core_of[nid] = c
        tile_of[nid] = t
        slot_of[nid] = 128 - cap[b]
        members[c][t].append(nid)
        cap[b] -= 1
        s += int(deg[nid])
        if cap[b] > 0:
            heapq.heappush(heap, (s, b))
    return core_of, tile_of, slot_of, members


def _preprocess(edge_src, edge_dst, edge_weight):
    src = np.asarray(edge_src, dtype=np.int64)
    dst = np.asarray(edge_dst, dtype=np.int64)
    wgt = np.asarray(edge_weight, dtype=np.float32)

    core_of, tile_of, slot_of, members = _balance_dsts(dst)
    core = core_of[dst]
    tile = tile_of[dst]

    counts = np.zeros((NC, TPC), dtype=np.int64)
    np.add.at(counts, (core, tile), 1)
    B = np.maximum((-(-counts // 128)).max(axis=0), 1)   # [TPC]
    NBLK = int(B.sum())
    blkoff = np.zeros(TPC, dtype=np.int64)
    off = 0
    for t in range(TPC):
        blkoff[t] = off
        off += int(B[t])
    BMAX = int(B.max())

    # gather components per tile: (tile, rel_block, nblocks, idx_col0)
    comps = []
    tile_comp0 = np.zeros(TPC + 1, dtype=np.int64)
    icol = 0
    for t in range(TPC):
        tile_comp0[t] = len(comps)
        rel = 0
        for nb in _pow2_components(int(B[t])):
            comps.append((t, rel, nb, icol))
            rel += nb
            icol += max(nb * 8, 64)   # 64-col (128B) aligned idx regions
        assert rel == int(B[t])
    tile_comp0[TPC] = len(comps)
    NICOL = icol
    # idx head/tail split: first few tiles' indices load in a small early
    # DMA so the first gather doesn't wait on the full idx table
    KCOL = comps[int(tile_comp0[6])][3]

    # per-core slot arrays
    idx_blk_all = []
    w_all = []
    dl_all = []
    qv_all = np.full((NC, NBLK * 128), -1, dtype=np.int8)
    for c in range(NC):
        sel = core == c
        t_c = tile[sel]
        s_c = src[sel]
        d_c = slot_of[dst[sel]]
        w_c = wgt[sel]
        q_c = (s_c & 3).astype(np.int8)
        o = np.lexsort((q_c, t_c))      # tile-major, q-minor
        t_c, s_c, d_c, w_c, q_c = t_c[o], s_c[o], d_c[o], w_c[o], q_c[o]

        idx_blk = np.zeros(NBLK * 128, dtype=np.int16)
        wf = np.zeros(NBLK * 128, dtype=np.float32)
        dlf = np.zeros(NBLK * 128, dtype=np.float32)
        cnt = counts[c]
        pos = 0
        for t in range(TPC):
            n = int(cnt[t])
            if n:
                slot0 = int(blkoff[t]) * 128
                idx_blk[slot0 : slot0 + n] = (s_c[pos : pos + n] >> 2)
                wf[slot0 : slot0 + n] = w_c[pos : pos + n]
                dlf[slot0 : slot0 + n] = d_c[pos : pos + n]
                qv_all[c, slot0 : slot0 + n] = q_c[pos : pos + n]
                pos += n
        assert pos == int(sel.sum())
        idx_blk_all.append(idx_blk)
        w_all.append(wf)
        dl_all.append(dlf)

    # pieces: per (tile, block): q values present in any core
    pieces = []            # (t, b, q, gb)
    tile_piece0 = np.zeros(TPC + 1, dtype=np.int64)
    for t in range(TPC):
        tile_piece0[t] = len(pieces)
        for b in range(int(B[t])):
            gb = int(blkoff[t]) + b
            qs = qv_all[:, gb * 128 : (gb + 1) * 128]
            present = np.unique(qs[qs >= 0])
            if len(present) == 0:
                present = [0]   # all-padding block (w=0 everywhere)
            for q in present:
                pieces.append((t, b, int(q), gb))
    tile_piece0[TPC] = len(pieces)
    NPIECE = len(pieces)

    per_core = []
    for c in range(NC):
        idx_blk = idx_blk_all[c]
        wf = w_all[c]
        dlf = dl_all[c]
        qv = qv_all[c]

        # idx storage: per component, 64-col-aligned regions (wrapped x8)
        idx_cols = np.zeros((NICOL, 16), dtype=np.int16)
        for (t, rel, nb, col0) in comps:
            blk0 = int(blkoff[t]) + rel
            flat = idx_blk[blk0 * 128 : (blk0 + nb) * 128]
            idx_cols[col0 : col0 + nb * 8] = flat.reshape(-1, 16)
        idx_w = np.tile(idx_cols.T, (8, 1)).copy()        # [128, NICOL]

        wp = np.zeros((NPIECE, 128), dtype=np.float32)
        for pid, (t, b, q, gb) in enumerate(pieces):
            s0 = gb * 128
            mask = qv[s0 : s0 + 128] == q
            wp[pid] = wf[s0 : s0 + 128] * mask
        wp = wp.T.copy()                                  # [128, NPIECE]
        dl_pb = dlf.reshape(NBLK, 128).T.copy()           # [128, NBLK]
        per_core.append((idx_w, wp, dl_pb))

    meta = dict(
        B=B, NBLK=NBLK, NICOL=NICOL, NPIECE=NPIECE, BMAX=BMAX, KCOL=KCOL,
        blkoff=blkoff, comps=comps, tile_comp0=tile_comp0,
        pieces=pieces, tile_piece0=tile_piece0,
        core_of=core_of, tile_of=tile_of, slot_of=slot_of,
    )
    return meta, per_core


def _build_program(meta):
    B = meta["B"]
    NBLK = meta["NBLK"]
    NICOL = meta["NICOL"]
    NPIECE = meta["NPIECE"]
    BMAX = meta["BMAX"]
    blkoff = meta["blkoff"]
    comps = meta["comps"]
    tile_comp0 = meta["tile_comp0"]
    pieces = meta["pieces"]
    tile_piece0 = meta["tile_piece0"]

    KCOL = meta["KCOL"]

    nc = bacc.Bacc()
    # x as quad rows: row r = x[4r : 4r+4] flattened (same bytes as [N, D])
    xq_d = nc.declare_dram_parameter("xq", [QROWS, QELEM], BF16, isOutput=False)
    idxh_d = nc.declare_dram_parameter("idxh", [128, KCOL], I16, isOutput=False)
    idx_d = nc.declare_dram_parameter("idx", [128, NICOL - KCOL], I16, isOutput=False)
    wp_d = nc.declare_dram_parameter("wp", [128, NPIECE], F32, isOutput=False)
    dl_d = nc.declare_dram_parameter("dl", [128, NBLK], F32, isOutput=False)
    iota_d = nc.declare_dram_parameter("iota", [128, 128], F32, isOutput=False)
    wn_d = nc.declare_dram_parameter("wn", [128, 128], BF16, isOutput=False)
    ws_d = nc.declare_dram_parameter("ws", [128, 128], BF16, isOutput=False)
    xt_d = nc.declare_dram_parameter("xt", [128, NPAD], BF16, isOutput=False)
    bias_d = nc.declare_dram_parameter("bias_bc", [128, 128], F32, isOutput=False)
    out_d = nc.declare_dram_parameter("out", [NPAD, 128], F32, isOutput=True)

    with TileContext(nc) as tc:
        with (
            tc.tile_pool(name="const", bufs=1) as cpool,
            tc.tile_pool(name="gather", bufs=4) as gpool,
            tc.tile_pool(name="work", bufs=8) as wpool,
            tc.tile_pool(name="outp", bufs=3) as opool,
            tc.tile_pool(name="psA", bufs=3, space="PSUM") as pApool,
            tc.tile_pool(name="psB", bufs=2, space="PSUM") as pBpool,
            tc.tile_pool(name="psI", bufs=1, space="PSUM") as pIpool,
        ):
            idxh_t = cpool.tile([128, KCOL], I16)
            nc.sync.dma_start(out=idxh_t[:], in_=idxh_d[:])
            idx_t = cpool.tile([128, NICOL - KCOL], I16)
            nc.sync.dma_start(out=idx_t[:], in_=idx_d[:])
            wp_t = cpool.tile([128, NPIECE], F32)
            nc.sync.dma_start(out=wp_t[:], in_=wp_d[:])
            dl_t = cpool.tile([128, NBLK], F32)
            nc.sync.dma_start(out=dl_t[:], in_=dl_d[:])
            iota_sb = cpool.tile([128, 128], F32)
            nc.sync.dma_start(out=iota_sb[:], in_=iota_d[:])
            wn_t = cpool.tile([128, 128], BF16)
            nc.sync.dma_start(out=wn_t[:], in_=wn_d[:])
            ws_t = cpool.tile([128, 128], BF16)
            nc.sync.dma_start(out=ws_t[:], in_=ws_d[:])
            xt_t = cpool.tile([128, NPAD], BF16)
            nc.sync.dma_start(out=xt_t[:], in_=xt_d[:])
            bias_t = cpool.tile([128, 128], F32)
            nc.sync.dma_start(out=bias_t[:], in_=bias_d[:])
            bias_bf = cpool.tile([1, 128], BF16)
            nc.vector.tensor_copy(out=bias_bf[:], in_=bias_t[0:1, :])
            ones_t = cpool.tile([1, 128], BF16)
            nc.vector.memset(ones_t[:], 1.0)

            iota_ps = pIpool.tile([128, 128], F32, space="PSUM")
            nc.vector.tensor_copy(out=iota_ps[:], in_=iota_sb[:])

            tile_gbuf = {}

            def issue_tile_gathers(t):
                gt = gpool.tile([128, BMAX * QELEM], BF16, tag="g")
                for ci in range(int(tile_comp0[t]), int(tile_comp0[t + 1])):
                    _, rel, nb, col0 = comps[ci]
                    if col0 < KCOL:
                        iap = idxh_t[:, col0 : col0 + nb * 8]
                    else:
                        iap = idx_t[:, col0 - KCOL : col0 - KCOL + nb * 8]
                    nc.gpsimd.dma_gather(
                        out_ap=gt[:, rel * QELEM : (rel + nb) * QELEM]
                        .rearrange("p (b e) -> p b e", e=QELEM),
                        in_ap=xq_d[:, :],
                        idxs_ap=iap,
                        num_idxs=nb * 128,
                        num_idxs_reg=nb * 128,
                        elem_size=QELEM,
                        single_packet=False,
                    )
                tile_gbuf[t] = gt

            for t in range(TPC):
                while len(tile_gbuf) < min(t + 4, TPC):
                    issue_tile_gathers(len(tile_gbuf))

                gt = tile_gbuf[t]
                p0, p1 = int(tile_piece0[t]), int(tile_piece0[t + 1])
                psumA = pApool.tile([128, 128], F32, space="PSUM", tag="psA")
                for i, pid in enumerate(range(p0, p1)):
                    _, b, q, gb = pieces[pid]
                    s_t = wpool.tile([128, 128], BF16, tag="sel")
                    nc.vector.tensor_scalar(
                        out=s_t[:],
                        in0=iota_ps[:],
                        scalar1=dl_t[:, gb : gb + 1],
                        scalar2=wp_t[:, pid : pid + 1],
                        op0=mybir.AluOpType.is_equal,
                        op1=mybir.AluOpType.mult,
                    )
                    nc.tensor.matmul(
                        out=psumA[:],
                        lhsT=gt[:, b * QELEM + q * 128 : b * QELEM + (q + 1) * 128],
                        rhs=s_t[:],
                        start=(i == 0),
                        stop=(pid == p1 - 1),
                    )
                aggT = wpool.tile([128, 128], BF16, tag="aggT")
                nc.scalar.copy(out=aggT[:], in_=psumA[:])
                psumB = pBpool.tile([128, 128], F32, space="PSUM", tag="psB")
                nc.tensor.matmul(
                    out=psumB[:], lhsT=aggT[:], rhs=wn_t[:],
                    start=True, stop=False,
                )
                nc.tensor.matmul(
                    out=psumB[:],
                    lhsT=xt_t[:, t * 128 : (t + 1) * 128],
                    rhs=ws_t[:],
                    start=False, stop=False,
                )
                nc.tensor.matmul(
                    out=psumB[:], lhsT=ones_t[:], rhs=bias_bf[:],
                    start=False, stop=True,
                )
                out_t = opool.tile([128, 128], F32, tag="out")
                nc.scalar.copy(out=out_t[:], in_=psumB[:])
                nc.sync.dma_start(
                    out=out_d[t * 128 : (t + 1) * 128, :], in_=out_t[:]
                )

    nc.compile()
    return nc


def kernel(x, edge_src, edge_dst, edge_weight, W_nbrs, W_self, bias, _trace=False,
           _tmpdir=None):
    x = np.asarray(x, dtype=np.float32)
    meta, per_core = _preprocess(edge_src, edge_dst, edge_weight)
    nc = _build_program(meta)

    xq = x.astype(nbf).reshape(QROWS, QELEM)
    iota = np.broadcast_to(np.arange(128, dtype=np.float32), (128, 128)).copy()
    wn = np.asarray(W_nbrs, dtype=np.float32).astype(nbf)
    ws = np.asarray(W_self, dtype=np.float32).astype(nbf)
    bias_bc = np.broadcast_to(np.asarray(bias, dtype=np.float32), (128, 128)).copy()

    core_of = meta["core_of"]
    tile_of = meta["tile_of"]
    slot_of = meta["slot_of"]
    in_maps = []
    for c in range(NC):
        idx_w, wp, dl_pb = per_core[c]
        nodes = np.where(core_of == c)[0]
        xt = np.zeros((128, NPAD), dtype=np.float32)
        xt[:, tile_of[nodes] * 128 + slot_of[nodes]] = x[nodes].T
        in_maps.append(
            dict(
                xq=xq,
                idxh=idx_w[:, : meta["KCOL"]].copy(),
                idx=idx_w[:, meta["KCOL"] :].copy(),
                wp=wp,
                dl=dl_pb,
                iota=iota,
                wn=wn,
                ws=ws,
                xt=xt.astype(nbf),
                bias_bc=bias_bc,
            )
        )

    res = run_bass_kernel_spmd(
        nc, in_maps, list(range(NC)), trace=_trace, tmpdir=_tmpdir
    )
    out = np.empty((N, D), dtype=np.float32)
    for c in range(NC):
        nodes = np.where(core_of == c)[0]
        out[nodes] = res.results[c]["out"][tile_of[nodes] * 128 + slot_of[nodes]]
    if _trace:
        kernel._last_result = res
    return out
